# revision 25
# baseline (speedup 1.0000x reference)
"""Trainium2 Bass kernel for segmented-LoRA linear (nn_Linear_73959336837249).

Math: out = x @ W.T + scale_g * ((x_g @ A_g.T) @ B_g.T), where the 16384
tokens form 4 contiguous segments of 4096, one adapter per segment.

Strategy:
  * Fold the LoRA update into the base weight per adapter on the host:
        Weff_g = W + s_g * B_g @ A_g        (exact algebraic identity)
    so each token segment needs a single dense matmul x_g @ Weff_g.T.
  * Shard tokens across the 8 NeuronCores (2048 tokens/core); each core's
    token range lives entirely inside one adapter segment, so each core
    gets exactly one [2048, 2048] effective weight.
  * On device: one big [2048 x 2048] @ [2048 x 2048] matmul per core,
    K-tiled over PSUM with 4 accumulation banks double-buffered.
  * Mixed precision on the contraction: the first 12 k-tiles (1536 dims)
    run in bf16 (1 col/cycle), the last 4 k-tiles (512 dims) run as two
    fp8(e4m3) DoubleRow pairs (2 k-planes per PE cell -> ~2x column rate).
    Measured rel err ~1.8e-2 vs the 2e-2 gate (bf16-only: 2.0e-3).
  * Output is stored bf16 (halves store traffic; adds ~1e-3 rel err in
    quadrature) and upcast to fp32 on the host.
  * Head scheduling: warm-up matmuls on a zeroed SBUF tile release the
    PE HAM clock-gate (1.2 GHz cold) during the DMA-bound head, and
    tiles 0-1 run fused k-major (with filler matmuls early) so the PE
    tracks the W-stream arrival rate instead of starving on it.

Self-contained: hardcodes all shapes; no file I/O.
"""

import numpy as np

# Problem shapes (hardcoded per contest contract)
N_ADAPTERS = 4
RANK = 16
D_IN = 2048
D_OUT = 2048
TOKENS = 16384
N_CORES = 8

T_LOC = TOKENS // N_CORES  # 2048 tokens per core
P = 128                    # partitions
KT = D_IN // P             # 16 contraction tiles
TT = T_LOC // P            # 16 token tiles per core
ON = 512                   # output-column tile (one PSUM bank of fp32)
NO = D_OUT // ON           # 4 o-tiles

# Number of 256-wide fp8 DoubleRow contraction pairs (0..2). Each pair
# covers 2 k-tiles. Remaining k-tiles run bf16.
FP8_PAIRS = 2
KB = KT - 2 * FP8_PAIRS    # bf16 k-tiles (first KB of KT)
WARMUP_MMS = 34
# Tiles 0..PAIR_TILES-1 run fused k-major: each arriving W k-tile feeds
# PAIR_TILES*NO matmuls, so compute (~1.7us/k) outpaces the W DMA stream
# (~1.3us/k) during the supply-bound head and the PE never starves.
PAIR_TILES = 2

_NC = {}


def _np_dtypes():
    import ml_dtypes

    return np.dtype(ml_dtypes.bfloat16), np.dtype(ml_dtypes.float8_e4m3)


def _build_nc():
    import concourse.mybir as mybir
    import concourse.tile as tile
    from concourse import bacc

    fp32 = mybir.dt.float32
    bf16 = mybir.dt.bfloat16
    fp8 = mybir.dt.float8e4

    nc = bacc.Bacc(None, target_bir_lowering=False)

    # xt[t, p, k*128+j] = x_tok[t*128+j, k*128+p]  (token-tile-major, d on
    # partitions), bf16 k-tiles only.
    xt = nc.dram_tensor("xt", [TT, P, KB * P], bf16, kind="ExternalInput")
    # wt[k, p, o] = Weff.T[k*128+p, o], bf16 k-tiles only.
    wt = nc.dram_tensor("wt", [KB, P, D_OUT], bf16, kind="ExternalInput")
    if FP8_PAIRS:
        # xt8[t, p, pair, s, m] = x_tok[t*128+m, KB*128 + pair*256 + s*128 + p]
        xt8 = nc.dram_tensor(
            "xt8", [TT, P, FP8_PAIRS, 2, P], fp8, kind="ExternalInput"
        )
        # wt8[pair, p, s, o] = Weff.T[KB*128 + pair*256 + s*128 + p, o]
        wt8 = nc.dram_tensor(
            "wt8", [FP8_PAIRS, P, 2, D_OUT], fp8, kind="ExternalInput"
        )
    out = nc.dram_tensor("out", [T_LOC, D_OUT], bf16, kind="ExternalOutput")

    with tile.TileContext(nc) as tc:
        with (
            tc.tile_pool(name="wpool", bufs=1) as wpool,
            tc.tile_pool(name="xpool", bufs=2) as xpool,
            tc.tile_pool(name="opool", bufs=3) as opool,
            tc.tile_pool(name="pspool", bufs=2, space="PSUM") as pspool,
        ):
            # --- urgent first chunks: x_0 k=0 slice + W_0 first o-chunk ---
            x_tiles = {}
            x_0 = xpool.tile([P, KB * P], bf16, tag="x", name="x_0")
            nc.scalar.dma_start(x_0[:, :P], xt[0, :, :P])
            w_tiles = [
                wpool.tile([P, D_OUT], bf16, tag=f"w{k}", name=f"w_{k}")
                for k in range(KB)
            ]
            nc.sync.dma_start(w_tiles[0][:, :ON], wt[0, :, :ON])

            # --- PE warm-up: release the HAM clock gate during the
            # DMA-bound head. Zeroed tile, results discarded. ---
            warm = wpool.tile([P, P], bf16, tag="warm", name="warm")
            nc.gpsimd.memset(warm[:], 0)
            ps_warm = pspool.tile([P, ON], fp32, tag="ps0", name="ps_warm")
            for i in range(WARMUP_MMS):
                nc.tensor.matmul(
                    ps_warm[:, :P], warm[:], warm[:], start=True, stop=True
                )

            # --- rest of x_0 and x_1 (the pair phase consumes both), then
            # the resident weight stream in consumption order. Few fat
            # DMAs: per-DMA overhead and the 8-deep completion-semaphore
            # recycling serialize the stream, so fragmenting loses more
            # than finer arrival wins; splitting W across both queues
            # measured WORSE (delays x_0's own k-slices). ---
            nc.scalar.dma_start(x_0[:, P:4 * P], xt[0, :, P:4 * P])
            nc.scalar.dma_start(x_0[:, 4 * P:], xt[0, :, 4 * P:])
            x_tiles[0] = x_0
            x8_tiles = {}
            x_1 = xpool.tile([P, KB * P], bf16, tag="x", name="x_1")
            nc.scalar.dma_start(x_1[:], xt[1])
            x_tiles[1] = x_1
            if FP8_PAIRS:
                x8_0 = xpool.tile(
                    [P, FP8_PAIRS, 2, P], fp8, tag="x8", name="x8_0"
                )
                nc.scalar.dma_start(x8_0[:], xt8[0])
                x8_tiles[0] = x8_0
                x8_1 = xpool.tile(
                    [P, FP8_PAIRS, 2, P], fp8, tag="x8", name="x8_1"
                )
                nc.scalar.dma_start(x8_1[:], xt8[1])
                x8_tiles[1] = x8_1

            nc.sync.dma_start(w_tiles[0][:, ON:], wt[0, :, ON:])
            for k in range(1, KB):
                nc.sync.dma_start(w_tiles[k][:], wt[k])
            w8_tiles = []
            for j in range(FP8_PAIRS):
                w8_j = wpool.tile([P, 2, D_OUT], fp8, tag=f"w8{j}", name=f"w8_{j}")
                nc.sync.dma_start(w8_j[:], wt8[j])
                w8_tiles.append(w8_j)

            def bf16_mms(t, ps, k, is_start, is_stop, start_k=0):
                lhsT = x_tiles[t][:, k * P:(k + 1) * P]  # [d, tok]
                for o in range(NO):
                    nc.tensor.matmul(
                        ps[o][:],
                        lhsT,
                        w_tiles[k][:, o * ON:(o + 1) * ON],
                        start=is_start and k == start_k,
                        stop=is_stop and k == KB - 1,
                    )

            def dr_mms(t, ps, j, is_start, is_stop):
                lhsT8 = x8_tiles[t][:, j]  # [d=128, s=2, tok=128]
                for o in range(NO):
                    nc.tensor.matmul(
                        ps[o][:],
                        lhsT8,
                        w8_tiles[j][:, :, o * ON:(o + 1) * ON],
                        start=is_start and j == 0,
                        stop=is_stop and j == FP8_PAIRS - 1,
                        perf_mode=mybir.MatmulPerfMode.DoubleRow,
                    )

            def drain(t, ps):
                # Copies split across DVE (o 0,1) and ACT (o 2,3) so the
                # two PSUM halves drain in parallel (different banks).
                o_t = opool.tile([P, D_OUT], bf16, tag="o", name=f"o_{t}")
                if t < TT - 1:
                    nc.vector.tensor_copy(o_t[:, 0 * ON:1 * ON], ps[0][:])
                    nc.vector.tensor_copy(o_t[:, 1 * ON:2 * ON], ps[1][:])
                    nc.scalar.copy(o_t[:, 2 * ON:3 * ON], ps[2][:])
                    nc.scalar.copy(o_t[:, 3 * ON:4 * ON], ps[3][:])
                    # stores ride the scalar queue so they never contend
                    # with the W stream on the sync queue
                    nc.scalar.dma_start(
                        out[t * P:(t + 1) * P, :D_OUT // 2], o_t[:, :D_OUT // 2]
                    )
                    nc.scalar.dma_start(
                        out[t * P:(t + 1) * P, D_OUT // 2:], o_t[:, D_OUT // 2:]
                    )
                else:
                    # last tile: per-o copies + stores, two parallel
                    # copy->store chains (DVE+sync / ACT+scalar) so the
                    # serial tail is minimal
                    for o in (0, 1):
                        nc.vector.tensor_copy(o_t[:, o * ON:(o + 1) * ON], ps[o][:])
                        nc.sync.dma_start(
                            out[t * P:(t + 1) * P, o * ON:(o + 1) * ON],
                            o_t[:, o * ON:(o + 1) * ON],
                        )
                    for o in (2, 3):
                        nc.scalar.copy(o_t[:, o * ON:(o + 1) * ON], ps[o][:])
                        nc.scalar.dma_start(
                            out[t * P:(t + 1) * P, o * ON:(o + 1) * ON],
                            o_t[:, o * ON:(o + 1) * ON],
                        )
                del x_tiles[t]
                if FP8_PAIRS:
                    del x8_tiles[t]

            # --- pair phase: tiles 0 and 1 fused k-major so the PE consumes
            # each arriving W k-tile with 8 matmuls and never starves during
            # the supply-bound head. Phased: (A) t0 alone for k<KH with
            # filler matmuls bridging the W-supply gaps (legal because t1
            # first touches its PSUM banks at k=KH, which clears them), then
            # (B) both tiles for k>=KH, (C) t1's k<KH, (D) the fp8 pairs. ---
            KH = 4
            pair_ps = {
                t: [
                    pspool.tile([P, ON], fp32, tag=f"ps{o}", name=f"ps_{t}_{o}")
                    for o in range(NO)
                ]
                for t in range(PAIR_TILES)
            }
            for k in range(KH):  # A
                bf16_mms(0, pair_ps[0], k, is_start=True, is_stop=False)
                for _ in range(4):  # keep the HAM clock gate open while
                    nc.tensor.matmul(  # the W stream ramps up
                        ps_warm[:], warm[:], x_tiles[0][:, :ON],
                        start=True, stop=True,
                    )
            for k in range(KH, KB):  # B
                bf16_mms(0, pair_ps[0], k, is_start=False, is_stop=False)
                bf16_mms(1, pair_ps[1], k, is_start=True, is_stop=False,
                         start_k=KH)
            for k in range(KH):  # C
                bf16_mms(1, pair_ps[1], k, is_start=False, is_stop=False)
            for j in range(FP8_PAIRS):  # D
                for t in range(PAIR_TILES):
                    dr_mms(t, pair_ps[t], j, is_start=False, is_stop=True)
            for t in range(PAIR_TILES):
                drain(t, pair_ps[t])

            # --- steady phase: resident W, one tile at a time ---
            for t in range(PAIR_TILES, TT):
                x_t = xpool.tile([P, KB * P], bf16, tag="x", name=f"x_{t}")
                if t == PAIR_TILES:
                    # just-in-time after the pair frees its ring slot:
                    # 3 chunks so k=0 can start while the rest streams
                    nc.scalar.dma_start(x_t[:, :4 * P], xt[t, :, :4 * P])
                    nc.scalar.dma_start(x_t[:, 4 * P:8 * P], xt[t, :, 4 * P:8 * P])
                    nc.scalar.dma_start(x_t[:, 8 * P:], xt[t, :, 8 * P:])
                else:
                    nc.scalar.dma_start(x_t[:], xt[t])
                x_tiles[t] = x_t
                if FP8_PAIRS:
                    x8_t = xpool.tile(
                        [P, FP8_PAIRS, 2, P], fp8, tag="x8", name=f"x8_{t}"
                    )
                    nc.scalar.dma_start(x8_t[:], xt8[t])
                    x8_tiles[t] = x8_t
                ps = [
                    pspool.tile([P, ON], fp32, tag=f"ps{o}", name=f"ps_{t}_{o}")
                    for o in range(NO)
                ]
                # Alternate DR-last (even t) / DR-first (odd t) so
                # consecutive tiles share the fp8<->bf16 PE mode switch
                # (2 switches per 2 tiles instead of 4).
                if FP8_PAIRS == 0:
                    for k in range(KB):
                        bf16_mms(t, ps, k, is_start=True, is_stop=True)
                elif t % 2 == 1:
                    for j in range(FP8_PAIRS):
                        dr_mms(t, ps, j, is_start=True, is_stop=False)
                    for k in range(KB):
                        bf16_mms(t, ps, k, is_start=False, is_stop=True)
                else:
                    for k in range(KB):
                        bf16_mms(t, ps, k, is_start=True, is_stop=False)
                    for j in range(FP8_PAIRS):
                        dr_mms(t, ps, j, is_start=False, is_stop=True)
                drain(t, ps)

    nc.compile()
    return nc


def _get_nc():
    if "nc" not in _NC:
        _NC["nc"] = _build_nc()
    return _NC["nc"]


def _prep_inputs(inputs):
    x = np.ascontiguousarray(np.asarray(inputs["x"], dtype=np.float32))
    W = np.asarray(inputs["W"], dtype=np.float32)
    lora_a = np.asarray(inputs["lora_a"], dtype=np.float32)
    lora_b = np.asarray(inputs["lora_b"], dtype=np.float32)
    scalings = np.asarray(inputs["scalings"], dtype=np.float32)
    bf16, fp8 = _np_dtypes()
    KBD = KB * P  # bf16 contraction dims

    # Fold LoRA into the transposed effective weight per adapter:
    # Weff.T = W.T + s * A.T @ B.T  -> [d_in, d_out]
    wts, wt8s = [], []
    for g in range(N_ADAPTERS):
        weff_t = W.T + scalings[g] * (lora_a[g].T @ lora_b[g].T)
        wts.append(
            np.ascontiguousarray(weff_t[:KBD].astype(bf16)).reshape(KB, P, D_OUT)
        )
        if FP8_PAIRS:
            # [pair, s, p, o] -> [pair, p, s, o]
            w8 = weff_t[KBD:].reshape(FP8_PAIRS, 2, P, D_OUT).transpose(0, 2, 1, 3)
            wt8s.append(np.ascontiguousarray(w8.astype(fp8)))

    in_maps = []
    for c in range(N_CORES):
        g = c * T_LOC // (TOKENS // N_ADAPTERS)
        xs = x[c * T_LOC:(c + 1) * T_LOC]  # [2048 tok, 2048 d]
        # [t, j, k, p] -> [t, p, k, j] -> [TT, 128, KB*128]
        xtl = np.ascontiguousarray(
            xs[:, :KBD].reshape(TT, P, KB, P).transpose(0, 3, 2, 1).astype(bf16)
        ).reshape(TT, P, KB * P)
        m = {"xt": xtl, "wt": wts[g]}
        if FP8_PAIRS:
            # [t, m, pair, s, p] -> [t, p, pair, s, m]
            x8 = (
                xs[:, KBD:]
                .reshape(TT, P, FP8_PAIRS, 2, P)
                .transpose(0, 4, 2, 3, 1)
            )
            m["xt8"] = np.ascontiguousarray(x8.astype(fp8))
            m["wt8"] = wt8s[g]
        in_maps.append(m)
    return in_maps


def _run(inputs, trace=False, **kwargs):
    from concourse.bass_utils import run_bass_kernel_spmd

    nc = _get_nc()
    in_maps = _prep_inputs(inputs)
    res = run_bass_kernel_spmd(
        nc, in_maps, core_ids=list(range(N_CORES)), trace=trace, **kwargs
    )
    out = np.concatenate(
        [r["out"].astype(np.float32) for r in res.results], axis=0
    )
    return out, res


def kernel(**inputs):
    out, _ = _run(inputs, trace=False)
    return out


# revision 27
# speedup vs baseline: 1.0029x; 1.0029x over previous
"""Trainium2 Bass kernel for segmented-LoRA linear (nn_Linear_73959336837249).

Math: out = x @ W.T + scale_g * ((x_g @ A_g.T) @ B_g.T), where the 16384
tokens form 4 contiguous segments of 4096, one adapter per segment.

Strategy:
  * Fold the LoRA update into the base weight per adapter on the host:
        Weff_g = W + s_g * B_g @ A_g        (exact algebraic identity)
    so each token segment needs a single dense matmul x_g @ Weff_g.T.
  * Shard tokens across the 8 NeuronCores (2048 tokens/core); each core's
    token range lives entirely inside one adapter segment, so each core
    gets exactly one [2048, 2048] effective weight.
  * On device: one big [2048 x 2048] @ [2048 x 2048] matmul per core,
    K-tiled over PSUM with 4 accumulation banks double-buffered.
  * Mixed precision on the contraction: the first 12 k-tiles (1536 dims)
    run in bf16 (1 col/cycle), the last 4 k-tiles (512 dims) run as two
    fp8(e4m3) DoubleRow pairs (2 k-planes per PE cell -> ~2x column rate).
    Measured rel err ~1.8e-2 vs the 2e-2 gate (bf16-only: 2.0e-3).
  * Output is stored bf16 (halves store traffic; adds ~1e-3 rel err in
    quadrature) and upcast to fp32 on the host.
  * Head scheduling: warm-up matmuls on a zeroed SBUF tile release the
    PE HAM clock-gate (1.2 GHz cold) during the DMA-bound head, and
    tiles 0-1 run fused k-major (with filler matmuls early) so the PE
    tracks the W-stream arrival rate instead of starving on it.

Self-contained: hardcodes all shapes; no file I/O.
"""

import numpy as np

# Problem shapes (hardcoded per contest contract)
N_ADAPTERS = 4
RANK = 16
D_IN = 2048
D_OUT = 2048
TOKENS = 16384
N_CORES = 8

T_LOC = TOKENS // N_CORES  # 2048 tokens per core
P = 128                    # partitions
KT = D_IN // P             # 16 contraction tiles
TT = T_LOC // P            # 16 token tiles per core
ON = 512                   # output-column tile (one PSUM bank of fp32)
NO = D_OUT // ON           # 4 o-tiles

# Number of 256-wide fp8 DoubleRow contraction pairs (0..2). Each pair
# covers 2 k-tiles. Remaining k-tiles run bf16.
FP8_PAIRS = 2
KB = KT - 2 * FP8_PAIRS    # bf16 k-tiles (first KB of KT)
WARMUP_MMS = 34
# Tiles 0..PAIR_TILES-1 run fused k-major: each arriving W k-tile feeds
# PAIR_TILES*NO matmuls, so compute (~1.7us/k) outpaces the W DMA stream
# (~1.3us/k) during the supply-bound head and the PE never starves.
PAIR_TILES = 2

_NC = {}


def _np_dtypes():
    import ml_dtypes

    return np.dtype(ml_dtypes.bfloat16), np.dtype(ml_dtypes.float8_e4m3)


def _build_nc():
    import concourse.mybir as mybir
    import concourse.tile as tile
    from concourse import bacc

    fp32 = mybir.dt.float32
    bf16 = mybir.dt.bfloat16
    fp8 = mybir.dt.float8e4

    nc = bacc.Bacc(None, target_bir_lowering=False)

    # xt[t, p, k*128+j] = x_tok[t*128+j, k*128+p]  (token-tile-major, d on
    # partitions), bf16 k-tiles only.
    xt = nc.dram_tensor("xt", [TT, P, KB * P], bf16, kind="ExternalInput")
    # wt[k, p, o] = Weff.T[k*128+p, o], bf16 k-tiles only.
    wt = nc.dram_tensor("wt", [KB, P, D_OUT], bf16, kind="ExternalInput")
    if FP8_PAIRS:
        # xt8[t, p, pair, s, m] = x_tok[t*128+m, KB*128 + pair*256 + s*128 + p]
        xt8 = nc.dram_tensor(
            "xt8", [TT, P, FP8_PAIRS, 2, P], fp8, kind="ExternalInput"
        )
        # wt8[pair, p, s, o] = Weff.T[KB*128 + pair*256 + s*128 + p, o]
        wt8 = nc.dram_tensor(
            "wt8", [FP8_PAIRS, P, 2, D_OUT], fp8, kind="ExternalInput"
        )
    out = nc.dram_tensor("out", [T_LOC, D_OUT], bf16, kind="ExternalOutput")

    with tile.TileContext(nc) as tc:
        with (
            tc.tile_pool(name="wpool", bufs=1) as wpool,
            tc.tile_pool(name="xpool", bufs=2) as xpool,
            tc.tile_pool(name="opool", bufs=3) as opool,
            tc.tile_pool(name="pspool", bufs=2, space="PSUM") as pspool,
        ):
            # --- urgent first chunks: x_0 k=0 slice + W_0 first o-chunk ---
            x_tiles = {}
            x_0 = xpool.tile([P, KB * P], bf16, tag="x", name="x_0")
            nc.scalar.dma_start(x_0[:, :P], xt[0, :, :P])
            w_tiles = [
                wpool.tile([P, D_OUT], bf16, tag=f"w{k}", name=f"w_{k}")
                for k in range(KB)
            ]
            nc.sync.dma_start(w_tiles[0][:, :ON], wt[0, :, :ON])

            # --- PE warm-up: release the HAM clock gate during the
            # DMA-bound head. Zeroed tile, results discarded. ---
            warm = wpool.tile([P, P], bf16, tag="warm", name="warm")
            nc.gpsimd.memset(warm[:], 0)
            ps_warm = pspool.tile([P, ON], fp32, tag="ps0", name="ps_warm")
            for i in range(WARMUP_MMS):
                nc.tensor.matmul(
                    ps_warm[:, :P], warm[:], warm[:], start=True, stop=True
                )

            # --- rest of x_0 and x_1 (the pair phase consumes both), then
            # the resident weight stream in consumption order. Few fat
            # DMAs: per-DMA overhead and the 8-deep completion-semaphore
            # recycling serialize the stream, so fragmenting loses more
            # than finer arrival wins; splitting W across both queues
            # measured WORSE (delays x_0's own k-slices). ---
            nc.scalar.dma_start(x_0[:, P:4 * P], xt[0, :, P:4 * P])
            nc.scalar.dma_start(x_0[:, 4 * P:], xt[0, :, 4 * P:])
            x_tiles[0] = x_0
            x8_tiles = {}
            x_1 = xpool.tile([P, KB * P], bf16, tag="x", name="x_1")
            nc.scalar.dma_start(x_1[:], xt[1])
            x_tiles[1] = x_1
            if FP8_PAIRS:
                x8_0 = xpool.tile(
                    [P, FP8_PAIRS, 2, P], fp8, tag="x8", name="x8_0"
                )
                nc.scalar.dma_start(x8_0[:], xt8[0])
                x8_tiles[0] = x8_0
                x8_1 = xpool.tile(
                    [P, FP8_PAIRS, 2, P], fp8, tag="x8", name="x8_1"
                )
                nc.scalar.dma_start(x8_1[:], xt8[1])
                x8_tiles[1] = x8_1

            nc.sync.dma_start(w_tiles[0][:, ON:], wt[0, :, ON:])
            for k in range(1, KB):
                nc.sync.dma_start(w_tiles[k][:], wt[k])
            w8_tiles = []
            for j in range(FP8_PAIRS):
                w8_j = wpool.tile([P, 2, D_OUT], fp8, tag=f"w8{j}", name=f"w8_{j}")
                nc.sync.dma_start(w8_j[:], wt8[j])
                w8_tiles.append(w8_j)

            def bf16_mms(t, ps, k, is_start, is_stop, start_k=0):
                lhsT = x_tiles[t][:, k * P:(k + 1) * P]  # [d, tok]
                for o in range(NO):
                    nc.tensor.matmul(
                        ps[o][:],
                        lhsT,
                        w_tiles[k][:, o * ON:(o + 1) * ON],
                        start=is_start and k == start_k,
                        stop=is_stop and k == KB - 1,
                    )

            def dr_mms(t, ps, j, is_start, is_stop):
                lhsT8 = x8_tiles[t][:, j]  # [d=128, s=2, tok=128]
                for o in range(NO):
                    nc.tensor.matmul(
                        ps[o][:],
                        lhsT8,
                        w8_tiles[j][:, :, o * ON:(o + 1) * ON],
                        start=is_start and j == 0,
                        stop=is_stop and j == FP8_PAIRS - 1,
                        perf_mode=mybir.MatmulPerfMode.DoubleRow,
                    )

            def drain(t, ps):
                # Copies split across DVE (o 0,1) and ACT (o 2,3) so the
                # two PSUM halves drain in parallel (different banks).
                o_t = opool.tile([P, D_OUT], bf16, tag="o", name=f"o_{t}")
                if t < TT - 1:
                    nc.vector.tensor_copy(o_t[:, 0 * ON:1 * ON], ps[0][:])
                    nc.vector.tensor_copy(o_t[:, 1 * ON:2 * ON], ps[1][:])
                    nc.scalar.copy(o_t[:, 2 * ON:3 * ON], ps[2][:])
                    nc.scalar.copy(o_t[:, 3 * ON:4 * ON], ps[3][:])
                    # stores ride the scalar queue so they never contend
                    # with the W stream on the sync queue
                    nc.scalar.dma_start(
                        out[t * P:(t + 1) * P, :D_OUT // 2], o_t[:, :D_OUT // 2]
                    )
                    nc.scalar.dma_start(
                        out[t * P:(t + 1) * P, D_OUT // 2:], o_t[:, D_OUT // 2:]
                    )
                else:
                    # last tile: per-o copies + stores, two parallel
                    # copy->store chains (DVE+sync / ACT+scalar) so the
                    # serial tail is minimal
                    for o in (0, 1):
                        nc.vector.tensor_copy(o_t[:, o * ON:(o + 1) * ON], ps[o][:])
                        nc.sync.dma_start(
                            out[t * P:(t + 1) * P, o * ON:(o + 1) * ON],
                            o_t[:, o * ON:(o + 1) * ON],
                        )
                    for o in (2, 3):
                        nc.scalar.copy(o_t[:, o * ON:(o + 1) * ON], ps[o][:])
                        nc.scalar.dma_start(
                            out[t * P:(t + 1) * P, o * ON:(o + 1) * ON],
                            o_t[:, o * ON:(o + 1) * ON],
                        )
                del x_tiles[t]
                if FP8_PAIRS:
                    del x8_tiles[t]

            # --- pair phase: tiles 0 and 1 fused k-major so the PE consumes
            # each arriving W k-tile with 8 matmuls and never starves during
            # the supply-bound head. Phased: (A) t0 alone for k<KH with
            # filler matmuls bridging the W-supply gaps (legal because t1
            # first touches its PSUM banks at k=KH, which clears them), then
            # (B) both tiles for k>=KH, (C) t1's k<KH, (D) the fp8 pairs. ---
            KH = 4
            pair_ps = {
                t: [
                    pspool.tile([P, ON], fp32, tag=f"ps{o}", name=f"ps_{t}_{o}")
                    for o in range(NO)
                ]
                for t in range(PAIR_TILES)
            }
            for k in range(KH):  # A
                bf16_mms(0, pair_ps[0], k, is_start=True, is_stop=False)
                # fillers keep the HAM clock gate open while the W stream
                # ramps up; later k-groups wait longer, so bridge more
                for _ in range(4 if k < 2 else 6):
                    nc.tensor.matmul(
                        ps_warm[:], warm[:], x_tiles[0][:, :ON],
                        start=True, stop=True,
                    )
            for k in range(KH, KB):  # B
                bf16_mms(0, pair_ps[0], k, is_start=False, is_stop=False)
                bf16_mms(1, pair_ps[1], k, is_start=True, is_stop=False,
                         start_k=KH)
            for k in range(KH):  # C
                bf16_mms(1, pair_ps[1], k, is_start=False, is_stop=False)
            for j in range(FP8_PAIRS):  # D
                for t in range(PAIR_TILES):
                    dr_mms(t, pair_ps[t], j, is_start=False, is_stop=True)
            for t in range(PAIR_TILES):
                drain(t, pair_ps[t])

            # --- steady phase: resident W, one tile at a time ---
            for t in range(PAIR_TILES, TT):
                x_t = xpool.tile([P, KB * P], bf16, tag="x", name=f"x_{t}")
                if t == PAIR_TILES:
                    # just-in-time after the pair frees its ring slot:
                    # 3 chunks so k=0 can start while the rest streams
                    nc.scalar.dma_start(x_t[:, :4 * P], xt[t, :, :4 * P])
                    nc.scalar.dma_start(x_t[:, 4 * P:8 * P], xt[t, :, 4 * P:8 * P])
                    nc.scalar.dma_start(x_t[:, 8 * P:], xt[t, :, 8 * P:])
                else:
                    nc.scalar.dma_start(x_t[:], xt[t])
                x_tiles[t] = x_t
                if FP8_PAIRS:
                    x8_t = xpool.tile(
                        [P, FP8_PAIRS, 2, P], fp8, tag="x8", name=f"x8_{t}"
                    )
                    nc.scalar.dma_start(x8_t[:], xt8[t])
                    x8_tiles[t] = x8_t
                ps = [
                    pspool.tile([P, ON], fp32, tag=f"ps{o}", name=f"ps_{t}_{o}")
                    for o in range(NO)
                ]
                # Alternate DR-last (even t) / DR-first (odd t) so
                # consecutive tiles share the fp8<->bf16 PE mode switch
                # (2 switches per 2 tiles instead of 4).
                if FP8_PAIRS == 0:
                    for k in range(KB):
                        bf16_mms(t, ps, k, is_start=True, is_stop=True)
                elif t == TT - 1:
                    # last tile: bf16 first, then the fp8 pairs o-OUTER so
                    # the four PSUM chains finish staggered and the final
                    # copy->store chains hide under the matmul shadow
                    for k in range(KB):
                        bf16_mms(t, ps, k, is_start=True, is_stop=False)
                    for o in range(NO):
                        for j in range(FP8_PAIRS):
                            nc.tensor.matmul(
                                ps[o][:],
                                x8_tiles[t][:, j],
                                w8_tiles[j][:, :, o * ON:(o + 1) * ON],
                                start=False,
                                stop=(j == FP8_PAIRS - 1),
                                perf_mode=mybir.MatmulPerfMode.DoubleRow,
                            )
                elif t % 2 == 1:
                    for j in range(FP8_PAIRS):
                        dr_mms(t, ps, j, is_start=True, is_stop=False)
                    for k in range(KB):
                        bf16_mms(t, ps, k, is_start=False, is_stop=True)
                else:
                    for k in range(KB):
                        bf16_mms(t, ps, k, is_start=True, is_stop=False)
                    for j in range(FP8_PAIRS):
                        dr_mms(t, ps, j, is_start=False, is_stop=True)
                drain(t, ps)

    nc.compile()
    return nc


def _get_nc():
    if "nc" not in _NC:
        _NC["nc"] = _build_nc()
    return _NC["nc"]


def _prep_inputs(inputs):
    x = np.ascontiguousarray(np.asarray(inputs["x"], dtype=np.float32))
    W = np.asarray(inputs["W"], dtype=np.float32)
    lora_a = np.asarray(inputs["lora_a"], dtype=np.float32)
    lora_b = np.asarray(inputs["lora_b"], dtype=np.float32)
    scalings = np.asarray(inputs["scalings"], dtype=np.float32)
    bf16, fp8 = _np_dtypes()
    KBD = KB * P  # bf16 contraction dims

    # Fold LoRA into the transposed effective weight per adapter:
    # Weff.T = W.T + s * A.T @ B.T  -> [d_in, d_out]
    wts, wt8s = [], []
    for g in range(N_ADAPTERS):
        weff_t = W.T + scalings[g] * (lora_a[g].T @ lora_b[g].T)
        wts.append(
            np.ascontiguousarray(weff_t[:KBD].astype(bf16)).reshape(KB, P, D_OUT)
        )
        if FP8_PAIRS:
            # [pair, s, p, o] -> [pair, p, s, o]
            w8 = weff_t[KBD:].reshape(FP8_PAIRS, 2, P, D_OUT).transpose(0, 2, 1, 3)
            wt8s.append(np.ascontiguousarray(w8.astype(fp8)))

    in_maps = []
    for c in range(N_CORES):
        g = c * T_LOC // (TOKENS // N_ADAPTERS)
        xs = x[c * T_LOC:(c + 1) * T_LOC]  # [2048 tok, 2048 d]
        # [t, j, k, p] -> [t, p, k, j] -> [TT, 128, KB*128]
        xtl = np.ascontiguousarray(
            xs[:, :KBD].reshape(TT, P, KB, P).transpose(0, 3, 2, 1).astype(bf16)
        ).reshape(TT, P, KB * P)
        m = {"xt": xtl, "wt": wts[g]}
        if FP8_PAIRS:
            # [t, m, pair, s, p] -> [t, p, pair, s, m]
            x8 = (
                xs[:, KBD:]
                .reshape(TT, P, FP8_PAIRS, 2, P)
                .transpose(0, 4, 2, 3, 1)
            )
            m["xt8"] = np.ascontiguousarray(x8.astype(fp8))
            m["wt8"] = wt8s[g]
        in_maps.append(m)
    return in_maps


def _run(inputs, trace=False, **kwargs):
    from concourse.bass_utils import run_bass_kernel_spmd

    nc = _get_nc()
    in_maps = _prep_inputs(inputs)
    res = run_bass_kernel_spmd(
        nc, in_maps, core_ids=list(range(N_CORES)), trace=trace, **kwargs
    )
    out = np.concatenate(
        [r["out"].astype(np.float32) for r in res.results], axis=0
    )
    return out, res


def kernel(**inputs):
    out, _ = _run(inputs, trace=False)
    return out


# revision 29
# speedup vs baseline: 1.0098x; 1.0068x over previous
"""Trainium2 Bass kernel for segmented-LoRA linear (nn_Linear_73959336837249).

Math: out = x @ W.T + scale_g * ((x_g @ A_g.T) @ B_g.T), where the 16384
tokens form 4 contiguous segments of 4096, one adapter per segment.

Strategy:
  * Fold the LoRA update into the base weight per adapter on the host:
        Weff_g = W + s_g * B_g @ A_g        (exact algebraic identity)
    so each token segment needs a single dense matmul x_g @ Weff_g.T.
  * Shard tokens across the 8 NeuronCores (2048 tokens/core); each core's
    token range lives entirely inside one adapter segment, so each core
    gets exactly one [2048, 2048] effective weight.
  * On device: one big [2048 x 2048] @ [2048 x 2048] matmul per core,
    K-tiled over PSUM with 4 accumulation banks double-buffered.
  * Mixed precision on the contraction: the first 12 k-tiles (1536 dims)
    run in bf16 (1 col/cycle), the last 4 k-tiles (512 dims) run as two
    fp8(e4m3) DoubleRow pairs (2 k-planes per PE cell -> ~2x column rate).
    Measured rel err ~1.8e-2 vs the 2e-2 gate (bf16-only: 2.0e-3).
  * Output is stored bf16 (halves store traffic; adds ~1e-3 rel err in
    quadrature) and upcast to fp32 on the host.
  * Head scheduling: warm-up matmuls on a zeroed SBUF tile release the
    PE HAM clock-gate (1.2 GHz cold) during the DMA-bound head, and
    tiles 0-1 run fused k-major (with filler matmuls early) so the PE
    tracks the W-stream arrival rate instead of starving on it.

Self-contained: hardcodes all shapes; no file I/O.
"""

import numpy as np

# Problem shapes (hardcoded per contest contract)
N_ADAPTERS = 4
RANK = 16
D_IN = 2048
D_OUT = 2048
TOKENS = 16384
N_CORES = 8

T_LOC = TOKENS // N_CORES  # 2048 tokens per core
P = 128                    # partitions
KT = D_IN // P             # 16 contraction tiles
TT = T_LOC // P            # 16 token tiles per core
ON = 512                   # output-column tile (one PSUM bank of fp32)
NO = D_OUT // ON           # 4 o-tiles

# Number of 256-wide fp8 DoubleRow contraction pairs (0..2). Each pair
# covers 2 k-tiles. Remaining k-tiles run bf16.
FP8_PAIRS = 2
KB = KT - 2 * FP8_PAIRS    # bf16 k-tiles (first KB of KT)
WARMUP_MMS = 34
# Tiles 0..PAIR_TILES-1 run fused k-major: each arriving W k-tile feeds
# PAIR_TILES*NO matmuls, so compute (~1.7us/k) outpaces the W DMA stream
# (~1.3us/k) during the supply-bound head and the PE never starves.
PAIR_TILES = 2

_NC = {}


def _np_dtypes():
    import ml_dtypes

    return np.dtype(ml_dtypes.bfloat16), np.dtype(ml_dtypes.float8_e4m3)


def _build_nc():
    import concourse.mybir as mybir
    import concourse.tile as tile
    from concourse import bacc

    fp32 = mybir.dt.float32
    bf16 = mybir.dt.bfloat16
    fp8 = mybir.dt.float8e4

    nc = bacc.Bacc(None, target_bir_lowering=False)

    # xt[t, p, k*128+j] = x_tok[t*128+j, k*128+p]  (token-tile-major, d on
    # partitions), bf16 k-tiles only.
    xt = nc.dram_tensor("xt", [TT, P, KB * P], bf16, kind="ExternalInput")
    # wt[k, p, o] = Weff.T[k*128+p, o], bf16 k-tiles only.
    wt = nc.dram_tensor("wt", [KB, P, D_OUT], bf16, kind="ExternalInput")
    if FP8_PAIRS:
        # xt8[t, p, pair, s, m] = x_tok[t*128+m, KB*128 + pair*256 + s*128 + p]
        xt8 = nc.dram_tensor(
            "xt8", [TT, P, FP8_PAIRS, 2, P], fp8, kind="ExternalInput"
        )
        # wt8[pair, p, s, o] = Weff.T[KB*128 + pair*256 + s*128 + p, o]
        wt8 = nc.dram_tensor(
            "wt8", [FP8_PAIRS, P, 2, D_OUT], fp8, kind="ExternalInput"
        )
    out = nc.dram_tensor("out", [T_LOC, D_OUT], bf16, kind="ExternalOutput")

    with tile.TileContext(nc) as tc:
        with (
            tc.tile_pool(name="wpool", bufs=1) as wpool,
            tc.tile_pool(name="xpool", bufs=2) as xpool,
            tc.tile_pool(name="opool", bufs=3) as opool,
            tc.tile_pool(name="pspool", bufs=2, space="PSUM") as pspool,
        ):
            # --- urgent first chunks: x_0 k=0 slice + W_0 first o-chunk ---
            x_tiles = {}
            x_0 = xpool.tile([P, KB * P], bf16, tag="x", name="x_0")
            nc.scalar.dma_start(x_0[:, :P], xt[0, :, :P])
            w_tiles = [
                wpool.tile([P, D_OUT], bf16, tag=f"w{k}", name=f"w_{k}")
                for k in range(KB)
            ]
            nc.sync.dma_start(w_tiles[0][:, :ON], wt[0, :, :ON])

            # --- PE warm-up: release the HAM clock gate during the
            # DMA-bound head. Zeroed tile, results discarded. ---
            warm = wpool.tile([P, P], bf16, tag="warm", name="warm")
            nc.gpsimd.memset(warm[:], 0)
            ps_warm = pspool.tile([P, ON], fp32, tag="ps0", name="ps_warm")
            for i in range(WARMUP_MMS):
                nc.tensor.matmul(
                    ps_warm[:, :P], warm[:], warm[:], start=True, stop=True
                )

            # --- rest of x_0 and x_1 (the pair phase consumes both), then
            # the resident weight stream in consumption order. Few fat
            # DMAs: per-DMA overhead and the 8-deep completion-semaphore
            # recycling serialize the stream, so fragmenting loses more
            # than finer arrival wins; splitting W across both queues
            # measured WORSE (delays x_0's own k-slices). ---
            nc.scalar.dma_start(x_0[:, P:4 * P], xt[0, :, P:4 * P])
            nc.scalar.dma_start(x_0[:, 4 * P:], xt[0, :, 4 * P:])
            x_tiles[0] = x_0
            x8_tiles = {}
            x_1 = xpool.tile([P, KB * P], bf16, tag="x", name="x_1")
            nc.scalar.dma_start(x_1[:], xt[1])
            x_tiles[1] = x_1
            if FP8_PAIRS:
                x8_0 = xpool.tile(
                    [P, FP8_PAIRS, 2, P], fp8, tag="x8", name="x8_0"
                )
                nc.scalar.dma_start(x8_0[:], xt8[0])
                x8_tiles[0] = x8_0
                x8_1 = xpool.tile(
                    [P, FP8_PAIRS, 2, P], fp8, tag="x8", name="x8_1"
                )
                nc.scalar.dma_start(x8_1[:], xt8[1])
                x8_tiles[1] = x8_1

            nc.sync.dma_start(w_tiles[0][:, ON:], wt[0, :, ON:])
            for k in range(1, KB):
                nc.sync.dma_start(w_tiles[k][:], wt[k])
            w8_tiles = []
            for j in range(FP8_PAIRS):
                w8_j = wpool.tile([P, 2, D_OUT], fp8, tag=f"w8{j}", name=f"w8_{j}")
                nc.sync.dma_start(w8_j[:], wt8[j])
                w8_tiles.append(w8_j)

            def bf16_mms(t, ps, k, is_start, is_stop, start_k=0):
                lhsT = x_tiles[t][:, k * P:(k + 1) * P]  # [d, tok]
                for o in range(NO):
                    nc.tensor.matmul(
                        ps[o][:],
                        lhsT,
                        w_tiles[k][:, o * ON:(o + 1) * ON],
                        start=is_start and k == start_k,
                        stop=is_stop and k == KB - 1,
                    )

            def dr_mms(t, ps, j, is_start, is_stop):
                lhsT8 = x8_tiles[t][:, j]  # [d=128, s=2, tok=128]
                for o in range(NO):
                    nc.tensor.matmul(
                        ps[o][:],
                        lhsT8,
                        w8_tiles[j][:, :, o * ON:(o + 1) * ON],
                        start=is_start and j == 0,
                        stop=is_stop and j == FP8_PAIRS - 1,
                        perf_mode=mybir.MatmulPerfMode.DoubleRow,
                    )

            def drain(t, ps):
                # Copies split across DVE (o 0,1) and ACT (o 2,3) so the
                # two PSUM halves drain in parallel (different banks).
                o_t = opool.tile([P, D_OUT], bf16, tag="o", name=f"o_{t}")
                if t < TT - 1:
                    nc.vector.tensor_copy(o_t[:, 0 * ON:1 * ON], ps[0][:])
                    nc.vector.tensor_copy(o_t[:, 1 * ON:2 * ON], ps[1][:])
                    nc.scalar.copy(o_t[:, 2 * ON:3 * ON], ps[2][:])
                    nc.scalar.copy(o_t[:, 3 * ON:4 * ON], ps[3][:])
                    # stores ride the scalar queue so they never contend
                    # with the W stream on the sync queue
                    nc.scalar.dma_start(
                        out[t * P:(t + 1) * P, :D_OUT // 2], o_t[:, :D_OUT // 2]
                    )
                    nc.scalar.dma_start(
                        out[t * P:(t + 1) * P, D_OUT // 2:], o_t[:, D_OUT // 2:]
                    )
                else:
                    # last tile: per-o copies + stores, parallel chains.
                    # o3 finishes last (staggered o-outer DR), so its
                    # copy/store go to vector+sync, which are idle by then;
                    # scalar handles only o2.
                    for o in (0, 1):
                        nc.vector.tensor_copy(o_t[:, o * ON:(o + 1) * ON], ps[o][:])
                        nc.sync.dma_start(
                            out[t * P:(t + 1) * P, o * ON:(o + 1) * ON],
                            o_t[:, o * ON:(o + 1) * ON],
                        )
                    nc.scalar.copy(o_t[:, 2 * ON:3 * ON], ps[2][:])
                    nc.scalar.dma_start(
                        out[t * P:(t + 1) * P, 2 * ON:3 * ON],
                        o_t[:, 2 * ON:3 * ON],
                    )
                    nc.vector.tensor_copy(o_t[:, 3 * ON:4 * ON], ps[3][:])
                    nc.sync.dma_start(
                        out[t * P:(t + 1) * P, 3 * ON:4 * ON],
                        o_t[:, 3 * ON:4 * ON],
                    )
                del x_tiles[t]
                if FP8_PAIRS:
                    del x8_tiles[t]

            # --- pair phase: tiles 0 and 1 fused k-major so the PE consumes
            # each arriving W k-tile with 8 matmuls and never starves during
            # the supply-bound head. Phased: (A) t0 alone for k<KH with
            # filler matmuls bridging the W-supply gaps (legal because t1
            # first touches its PSUM banks at k=KH, which clears them), then
            # (B) both tiles for k>=KH, (C) t1's k<KH, (D) the fp8 pairs. ---
            KH = 4
            pair_ps = {
                t: [
                    pspool.tile([P, ON], fp32, tag=f"ps{o}", name=f"ps_{t}_{o}")
                    for o in range(NO)
                ]
                for t in range(PAIR_TILES)
            }
            for k in range(KH):  # A
                bf16_mms(0, pair_ps[0], k, is_start=True, is_stop=False)
                # fillers keep the HAM clock gate open while the W stream
                # ramps up (the longest observed supply stall is ~3.2us,
                # within 200ns of the ~3.4us re-throttle window)
                for _ in range(6):
                    nc.tensor.matmul(
                        ps_warm[:], warm[:], x_tiles[0][:, :ON],
                        start=True, stop=True,
                    )
            for k in range(KH, KB):  # B
                bf16_mms(0, pair_ps[0], k, is_start=False, is_stop=False)
                bf16_mms(1, pair_ps[1], k, is_start=True, is_stop=False,
                         start_k=KH)
            for k in range(KH):  # C
                bf16_mms(1, pair_ps[1], k, is_start=False, is_stop=False)
            for j in range(FP8_PAIRS):  # D
                for t in range(PAIR_TILES):
                    dr_mms(t, pair_ps[t], j, is_start=False, is_stop=True)
            for t in range(PAIR_TILES):
                drain(t, pair_ps[t])

            # --- steady phase: resident W, one tile at a time ---
            for t in range(PAIR_TILES, TT):
                x_t = xpool.tile([P, KB * P], bf16, tag="x", name=f"x_{t}")
                if t == PAIR_TILES:
                    # just-in-time after the pair frees its ring slot:
                    # 3 chunks so k=0 can start while the rest streams
                    nc.scalar.dma_start(x_t[:, :4 * P], xt[t, :, :4 * P])
                    nc.scalar.dma_start(x_t[:, 4 * P:8 * P], xt[t, :, 4 * P:8 * P])
                    nc.scalar.dma_start(x_t[:, 8 * P:], xt[t, :, 8 * P:])
                else:
                    nc.scalar.dma_start(x_t[:], xt[t])
                x_tiles[t] = x_t
                if FP8_PAIRS:
                    x8_t = xpool.tile(
                        [P, FP8_PAIRS, 2, P], fp8, tag="x8", name=f"x8_{t}"
                    )
                    nc.scalar.dma_start(x8_t[:], xt8[t])
                    x8_tiles[t] = x8_t
                ps = [
                    pspool.tile([P, ON], fp32, tag=f"ps{o}", name=f"ps_{t}_{o}")
                    for o in range(NO)
                ]
                # Alternate DR-last (even t) / DR-first (odd t) so
                # consecutive tiles share the fp8<->bf16 PE mode switch
                # (2 switches per 2 tiles instead of 4).
                if FP8_PAIRS == 0:
                    for k in range(KB):
                        bf16_mms(t, ps, k, is_start=True, is_stop=True)
                elif t == TT - 1:
                    # last tile: bf16 first, then the fp8 pairs o-OUTER so
                    # the four PSUM chains finish staggered and the final
                    # copy->store chains hide under the matmul shadow
                    for k in range(KB):
                        bf16_mms(t, ps, k, is_start=True, is_stop=False)
                    for o in range(NO):
                        for j in range(FP8_PAIRS):
                            nc.tensor.matmul(
                                ps[o][:],
                                x8_tiles[t][:, j],
                                w8_tiles[j][:, :, o * ON:(o + 1) * ON],
                                start=False,
                                stop=(j == FP8_PAIRS - 1),
                                perf_mode=mybir.MatmulPerfMode.DoubleRow,
                            )
                elif t % 2 == 1:
                    for j in range(FP8_PAIRS):
                        dr_mms(t, ps, j, is_start=True, is_stop=False)
                    for k in range(KB):
                        bf16_mms(t, ps, k, is_start=False, is_stop=True)
                else:
                    for k in range(KB):
                        bf16_mms(t, ps, k, is_start=True, is_stop=False)
                    for j in range(FP8_PAIRS):
                        dr_mms(t, ps, j, is_start=False, is_stop=True)
                drain(t, ps)

    nc.compile()
    return nc


def _get_nc():
    if "nc" not in _NC:
        _NC["nc"] = _build_nc()
    return _NC["nc"]


def _prep_inputs(inputs):
    x = np.ascontiguousarray(np.asarray(inputs["x"], dtype=np.float32))
    W = np.asarray(inputs["W"], dtype=np.float32)
    lora_a = np.asarray(inputs["lora_a"], dtype=np.float32)
    lora_b = np.asarray(inputs["lora_b"], dtype=np.float32)
    scalings = np.asarray(inputs["scalings"], dtype=np.float32)
    bf16, fp8 = _np_dtypes()
    KBD = KB * P  # bf16 contraction dims

    # Fold LoRA into the transposed effective weight per adapter:
    # Weff.T = W.T + s * A.T @ B.T  -> [d_in, d_out]
    wts, wt8s = [], []
    for g in range(N_ADAPTERS):
        weff_t = W.T + scalings[g] * (lora_a[g].T @ lora_b[g].T)
        wts.append(
            np.ascontiguousarray(weff_t[:KBD].astype(bf16)).reshape(KB, P, D_OUT)
        )
        if FP8_PAIRS:
            # [pair, s, p, o] -> [pair, p, s, o]
            w8 = weff_t[KBD:].reshape(FP8_PAIRS, 2, P, D_OUT).transpose(0, 2, 1, 3)
            wt8s.append(np.ascontiguousarray(w8.astype(fp8)))

    in_maps = []
    for c in range(N_CORES):
        g = c * T_LOC // (TOKENS // N_ADAPTERS)
        xs = x[c * T_LOC:(c + 1) * T_LOC]  # [2048 tok, 2048 d]
        # [t, j, k, p] -> [t, p, k, j] -> [TT, 128, KB*128]
        xtl = np.ascontiguousarray(
            xs[:, :KBD].reshape(TT, P, KB, P).transpose(0, 3, 2, 1).astype(bf16)
        ).reshape(TT, P, KB * P)
        m = {"xt": xtl, "wt": wts[g]}
        if FP8_PAIRS:
            # [t, m, pair, s, p] -> [t, p, pair, s, m]
            x8 = (
                xs[:, KBD:]
                .reshape(TT, P, FP8_PAIRS, 2, P)
                .transpose(0, 4, 2, 3, 1)
            )
            m["xt8"] = np.ascontiguousarray(x8.astype(fp8))
            m["wt8"] = wt8s[g]
        in_maps.append(m)
    return in_maps


def _run(inputs, trace=False, **kwargs):
    from concourse.bass_utils import run_bass_kernel_spmd

    nc = _get_nc()
    in_maps = _prep_inputs(inputs)
    res = run_bass_kernel_spmd(
        nc, in_maps, core_ids=list(range(N_CORES)), trace=trace, **kwargs
    )
    out = np.concatenate(
        [r["out"].astype(np.float32) for r in res.results], axis=0
    )
    return out, res


def kernel(**inputs):
    out, _ = _run(inputs, trace=False)
    return out
